# revision 16
# baseline (speedup 1.0000x reference)
"""Cosine attention kernel for Trainium2, sharded over 8 NeuronCores.

Problem: N=4, L=S=2048, H=8, D=64 fp32.
  q = queries / ||queries||_D ; k = keys / ||keys||_D
  qk = einsum('nlhd,nshd->nlsh', q, k); A = softmax(qk / temp, axis=2)
  out = einsum('nlsh,nshd->nlhd', A, values)

Sharding: the 32 (n, h) pairs are split 4-per-core (data + head parallel).
Each core computes 4 independent 2048x2048 attention problems.

Per-core schedule (v10): ACT's softmax Exp over the full 2048x2048 score
matrix (128 x [128,1024] activations, ~1.11us each) is the critical
resource; the PE (bf16 matmuls ~226ns/512col warm) runs underneath it.
The PE's HAM clock gate needs sustained busy-ness: an under-utilized PE
oscillates between 1.2/2.4 GHz for the first ~60us.  Pair 0's score
matmuls therefore use f32r operands (330ns/512col): 2x330 + 2x226 =
1112ns/s-tile = the ACT cadence, so the PE runs at ~100% duty through
the HAM danger zone and the clock stays at 2.4GHz; pairs 1-3 switch to
all-bf16 (84% duty, safe once warm).  Per-pair prep (row norms on DVE,
rsqrt via one Ln+Exp ACT pair -- same act-table set as the softmax Exp,
preloaded at start -- normalize, PE transposes, V load+cast) is
software-pipelined under the previous pair's main loop.

Main loop per (pair, 1024-col L-chunk):
  - P^T[s_tile, l] = KnT_tile^T @ QnT into PSUM [128,1024] (2 matmuls)
  - pexp = Exp(P^T) on ACT -> SBUF bf16
  - psum2[h][65, 512] += V_aug[s_tile]^T @ pexp_half (bf16, 16-step accum;
    row 64 accumulates the softmax denominator via V's ones column)
  - epilogue per 512-half: DVE copy -> PE transpose [65,128] -> [128,65] ->
    DVE reciprocal + scale -> one chunked DMA [128,4,64] per half.
"""

import sys

if "/opt/trn_rl_repo" not in sys.path:
    sys.path.insert(0, "/opt/trn_rl_repo")

import numpy as np

N_CORES = 8
PAIRS = 4          # (n, h) pairs per core
L = 2048           # query length
S = 2048           # key length
D = 64             # head dim
T = S // 128       # 128-row tiles per pair
LC = 2             # L chunks
LCHUNK = L // LC   # 1024

_PROGRAM_CACHE = {}


def _build_program():
    import concourse.tile as tile
    import concourse.bass as bass
    from concourse import bacc, mybir
    from concourse.bass import ds
    from concourse.masks import make_identity

    f32 = mybir.dt.float32
    f32r = mybir.dt.float32r
    bf16 = mybir.dt.bfloat16
    AF = mybir.ActivationFunctionType

    nc = bacc.Bacc("TRN2", target_bir_lowering=False, debug=False,
                   num_devices=N_CORES)
    q_hbm = nc.dram_tensor("q", [PAIRS, L, D], f32, kind="ExternalInput")
    k_hbm = nc.dram_tensor("k", [PAIRS, S, D], f32, kind="ExternalInput")
    v_hbm = nc.dram_tensor("v", [PAIRS, S, D], f32, kind="ExternalInput")
    t_hbm = nc.dram_tensor("temp", [1, 1], f32, kind="ExternalInput")
    o_hbm = nc.dram_tensor("o", [PAIRS, L, D], f32, kind="ExternalOutput")

    # Per-pair mm1 operand dtype: f32r for pair 0 (PE duty ~100% keeps the
    # HAM un-throttled through the cold-start zone), bf16 after.
    MM1_DT = {0: bf16, 1: bf16, 2: bf16, 3: bf16}

    with tile.TileContext(nc) as tc:
        with (
            tc.tile_pool(name="const", bufs=1) as cpool,
            tc.tile_pool(name="raw", bufs=1) as raw_pool,
            tc.tile_pool(name="io", bufs=2) as io_pool,
            tc.tile_pool(name="work", bufs=2) as work_pool,
            tc.tile_pool(name="small", bufs=4) as small_pool,
            tc.tile_pool(name="pexp", bufs=6) as pexp_pool,
            tc.tile_pool(name="psum1", bufs=2, space="PSUM") as psum1_pool,
            tc.tile_pool(name="psum2", bufs=2, space="PSUM") as psum2_pool,
            tc.tile_pool(name="psmall", bufs=2, space="PSUM") as psmall_pool,
            tc.tile_pool(name="dram", bufs=1, space="DRAM") as dram_pool,
        ):
            identity = cpool.tile([128, 128], f32)
            make_identity(nc, identity[:])
            identity_bf = cpool.tile([128, 128], bf16)
            nc.vector.tensor_copy(identity_bf[:], identity[:])
            identity_r = cpool.tile([128, 128], f32r)
            nc.vector.tensor_copy(identity_r[:], identity[:])

            # Preload the Ln/Exp activation table set off the critical path:
            # no table load between the first norm chain and the softmax Exp.
            tbl = cpool.tile([128, 1], f32)
            nc.vector.memset(tbl[:], 1.0)
            nc.scalar.activation(tbl[:], tbl[:], AF.Ln)

            # Warm-keeper ingredients: regular matmuls count as HAM
            # activity (transpose-mode does not), keeping the PE p-state up.
            scratch_f = cpool.tile([128, 512], f32)
            nc.vector.memset(scratch_f[:], 0.0)
            scratch_b = cpool.tile([128, 512], bf16)
            nc.vector.tensor_copy(scratch_b[:], scratch_f[:])
            scratch_w = cpool.tile([128, 128], bf16)
            nc.vector.memset(scratch_w[:], 0.0)

            def warm(n):
                # fresh pool tiles each time: never pins a psmall slot
                for i in range(n):
                    wk = psmall_pool.tile([128, 512], f32, tag="tp", name="wk")
                    nc.tensor.matmul(wk[:], scratch_w[:], scratch_b[:])

            # HAM warmup while input DMAs stream in.
            warm(12)

            # 1/temp broadcast to [128, 1] (bounce through DRAM for the
            # partition-broadcast DMA).
            t_sb = cpool.tile([1, 1], f32)
            nc.sync.dma_start(t_sb[:], t_hbm.ap())
            rt_sb = cpool.tile([1, 1], f32)
            nc.vector.reciprocal(rt_sb[:], t_sb[:])
            rt_dram = dram_pool.tile([1, 1], f32)
            nc.sync.dma_start(rt_dram[:], rt_sb[:])
            rt_b = cpool.tile([128, 1], f32)
            nc.sync.dma_start(rt_b[:], rt_dram[:].to_broadcast([128, 1]))

            q_raw, k_raw = {}, {}
            v_stage, v_aug = {}, {}

            def dma_qk(p):
                q_raw[p] = raw_pool.tile([128, T, D], f32,
                                         tag=f"qraw{p}", name=f"qraw{p}")
                nc.sync.dma_start(
                    q_raw[p][:],
                    q_hbm.ap()[p].rearrange("(t pp) d -> pp t d", pp=128))
                k_raw[p] = raw_pool.tile([128, T, D], f32,
                                         tag=f"kraw{p}", name=f"kraw{p}")
                nc.sync.dma_start(
                    k_raw[p][:],
                    k_hbm.ap()[p].rearrange("(t pp) d -> pp t d", pp=128))

            def prep_v(p):
                # V with ones column appended; bf16 for mm2.
                v_stage[p] = io_pool.tile([128, T, D + 1], f32, tag="vstage",
                                          name=f"vstage{p}")
                nc.vector.memset(v_stage[p][:, :, D:D + 1], 1.0)
                nc.gpsimd.dma_start(
                    v_stage[p][:, :, 0:D],
                    v_hbm.ap()[p].rearrange("(t pp) d -> pp t d", pp=128))
                v_aug[p] = io_pool.tile([128, T, D + 1], bf16, tag="vaug",
                                        name=f"vaug{p}")
                nc.vector.tensor_copy(v_aug[p][:], v_stage[p][:])

            dma_qk(0)
            prep_v(0)
            for p in range(1, PAIRS):
                dma_qk(p)

            ssq, r_n, rq, rk = {}, {}, {}, {}

            def prep_norm(p):
                # Row sum-of-squares on DVE; rsqrt as exp(-0.5*ln(ssq)).
                ssq[p] = cpool.tile([128, 2, T], f32, name=f"ssq{p}",
                                    tag=f"ssq{p}")
                for i, srct in ((0, q_raw[p]), (1, k_raw[p])):
                    sq = work_pool.tile([128, T, D], f32, tag="sq")
                    nc.vector.tensor_mul(sq[:], srct[:], srct[:])
                    nc.vector.tensor_reduce(
                        ssq[p][:, i, :], sq[:],
                        axis=mybir.AxisListType.X, op=mybir.AluOpType.add)
                r_n[p] = cpool.tile([128, 2, T], f32, name=f"r_n{p}",
                                    tag=f"r_n{p}")
                nc.scalar.activation(ssq[p][:], ssq[p][:], AF.Ln)
                nc.scalar.activation(r_n[p][:], ssq[p][:], AF.Exp, scale=-0.5)
                rq[p] = r_n[p][:, 0, :]
                rk[p] = r_n[p][:, 1, :]
                nc.vector.tensor_scalar_mul(rk[p], rk[p], rt_b[:])

            qnT, knT = {}, {}

            def prep_tp(p):
                # Normalize + PE-transpose to [64, 2048] in the pair's mm1
                # dtype (f32r for pair 0, bf16 after).
                dt = MM1_DT[p]
                ident = identity_r if dt == f32r else identity_bf
                qn = work_pool.tile([128, T, D], dt, tag="qn")
                kn = work_pool.tile([128, T, D], dt, tag="kn")
                for rr, srct, dstt in ((rq[p], q_raw[p], qn),
                                       (rk[p], k_raw[p], kn)):
                    r_b = bass.AP(tensor=rr.tensor, offset=rr.offset,
                                  ap=[rr.ap[0], rr.ap[1], [0, D]])
                    nc.vector.tensor_mul(dstt[:], srct[:], r_b)
                qnT[p] = raw_pool.tile([64, L], dt, tag=f"qnT{p}",
                                       name=f"qnT{p}")
                knT[p] = raw_pool.tile([64, S], dt, tag=f"knT{p}",
                                       name=f"knT{p}")
                for srct, dstt in ((qn, qnT[p]), (kn, knT[p])):
                    for g in range(T // 4):
                        tp = psmall_pool.tile([64, 4, 128], dt, tag="tp")
                        for j in range(4):
                            nc.tensor.transpose(
                                tp[:, j, :], srct[:, 4 * g + j, :], ident)
                        nc.vector.tensor_copy(dstt[:, ds(512 * g, 512)], tp[:])
                    warm(1)  # keep the HAM busy-window alive

            def main_chunk(p, lc):
                ps2 = {}
                for h in range(LCHUNK // 512):
                    ps2[h] = psum2_pool.tile([D + 1, 512], f32, tag="ps2",
                                             name=f"ps2_{h}")
                for st in range(T):
                    ps1 = psum1_pool.tile([128, LCHUNK], f32, tag="ps1")
                    lhs1 = knT[p][:, ds(st * 128, 128)]
                    for h in range(LCHUNK // 512):
                        nc.tensor.matmul(
                            ps1[:, ds(h * 512, 512)], lhs1,
                            qnT[p][:, ds(lc * LCHUNK + h * 512, 512)])
                    pexp = pexp_pool.tile([128, LCHUNK], bf16, tag="pexp")
                    nc.scalar.activation(pexp[:], ps1[:], AF.Exp)
                    lhs2 = v_aug[p][:, st, :]
                    for h in range(LCHUNK // 512):
                        nc.tensor.matmul(
                            ps2[h][:], lhs2,
                            pexp[:, ds(h * 512, 512)],
                            start=(st == 0), stop=(st == T - 1))

                # Epilogue per 512-half; per-128 copy slices so the
                # copy->transpose->scale chain pipelines, one chunked DMA
                # per half.
                for h in range(LCHUNK // 512):
                    o_sb = work_pool.tile([D + 1, 512], f32, tag="osb")
                    o_fin = small_pool.tile([128, 4, D], f32, tag="ofin")
                    for j in range(512 // 128):
                        nc.vector.tensor_copy(
                            o_sb[:, ds(j * 128, 128)],
                            ps2[h][:, ds(j * 128, 128)])
                        tp = psmall_pool.tile([128, D + 1], f32, tag="tp")
                        nc.tensor.transpose(
                            tp[:], o_sb[:, ds(j * 128, 128)],
                            identity[0:D + 1, 0:D + 1])
                        rcp = small_pool.tile([128, 1], f32, tag="rcp")
                        nc.vector.reciprocal(rcp[:], tp[:, D:D + 1])
                        nc.vector.tensor_scalar_mul(
                            o_fin[:, j, :], tp[:, 0:D], rcp[:])
                    nc.sync.dma_start(
                        o_hbm.ap()[p, ds(lc * LCHUNK + h * 512, 512), :]
                            .rearrange("(j pp) d -> pp j d", pp=128),
                        o_fin[:])

            # ---- Pipelined schedule: pair p+1's prep runs under pair p's
            # main loop; only pair 0's prep is exposed.
            prep_norm(0)
            prep_tp(0)
            prep_norm(1)
            main_chunk(0, 0)
            prep_tp(1)
            prep_v(1)
            main_chunk(0, 1)
            prep_norm(2)
            main_chunk(1, 0)
            prep_tp(2)
            prep_v(2)
            main_chunk(1, 1)
            prep_norm(3)
            main_chunk(2, 0)
            prep_tp(3)
            prep_v(3)
            main_chunk(2, 1)
            main_chunk(3, 0)
            main_chunk(3, 1)

    nc.compile()
    return nc


def _get_program():
    if "nc" not in _PROGRAM_CACHE:
        _PROGRAM_CACHE["nc"] = _build_program()
    return _PROGRAM_CACHE["nc"]


def kernel(queries, keys, values, temp_scale):
    from concourse.bass_utils import run_bass_kernel_spmd

    N, Lq, H, Dh = queries.shape
    assert (N, Lq, H, Dh) == (4, L, 8, D), (N, Lq, H, Dh)

    # [N, L, H, D] -> [N*H, L, D]; core c owns pairs 4c..4c+4.
    def shard(x):
        x = np.ascontiguousarray(
            np.asarray(x, dtype=np.float32).transpose(0, 2, 1, 3)
        ).reshape(N * H, Lq, Dh)
        return [np.ascontiguousarray(x[PAIRS * c:PAIRS * (c + 1)])
                for c in range(N_CORES)]

    qs, ks, vs = shard(queries), shard(keys), shard(values)
    t11 = np.asarray(temp_scale, dtype=np.float32).reshape(1, 1)
    in_maps = [
        {"q": qs[c], "k": ks[c], "v": vs[c], "temp": t11}
        for c in range(N_CORES)
    ]

    nc = _get_program()
    res = run_bass_kernel_spmd(nc, in_maps, core_ids=list(range(N_CORES)))
    if getattr(res, "exec_time_ns", None):
        print(f"HW exec time: {res.exec_time_ns} ns")

    out = np.stack([res.results[c]["o"] for c in range(N_CORES)])  # [8,4,L,D]
    out = out.reshape(N, H, Lq, Dh).transpose(0, 2, 1, 3)          # [N,L,H,D]
    return np.ascontiguousarray(out)


# revision 17
# speedup vs baseline: 1.0548x; 1.0548x over previous
"""Cosine attention kernel for Trainium2, sharded over 8 NeuronCores.

Problem: N=4, L=S=2048, H=8, D=64 fp32.
  q = queries / ||queries||_D ; k = keys / ||keys||_D
  qk = einsum('nlhd,nshd->nlsh', q, k); A = softmax(qk / temp, axis=2)
  out = einsum('nlsh,nshd->nlhd', A, values)

Sharding: the 32 (n, h) pairs are split 4-per-core (data + head parallel).
Each core computes 4 independent 2048x2048 attention problems.

Per-core schedule (v10): ACT's softmax Exp over the full 2048x2048 score
matrix (128 x [128,1024] activations, ~1.11us each) is the critical
resource; the PE (bf16 matmuls ~226ns/512col warm) runs underneath it.
The PE's HAM clock gate needs sustained busy-ness: an under-utilized PE
oscillates between 1.2/2.4 GHz for the first ~60us.  Pair 0's score
matmuls therefore use f32r operands (330ns/512col): 2x330 + 2x226 =
1112ns/s-tile = the ACT cadence, so the PE runs at ~100% duty through
the HAM danger zone and the clock stays at 2.4GHz; pairs 1-3 switch to
all-bf16 (84% duty, safe once warm).  Per-pair prep (row norms on DVE,
rsqrt via one Ln+Exp ACT pair -- same act-table set as the softmax Exp,
preloaded at start -- normalize, PE transposes, V load+cast) is
software-pipelined under the previous pair's main loop.

Main loop per (pair, 1024-col L-chunk):
  - P^T[s_tile, l] = KnT_tile^T @ QnT into PSUM [128,1024] (2 matmuls)
  - pexp = Exp(P^T) on ACT -> SBUF bf16
  - psum2[h][65, 512] += V_aug[s_tile]^T @ pexp_half (bf16, 16-step accum;
    row 64 accumulates the softmax denominator via V's ones column)
  - epilogue per 512-half: DVE copy -> PE transpose [65,128] -> [128,65] ->
    DVE reciprocal + scale -> one chunked DMA [128,4,64] per half.
"""

import sys

if "/opt/trn_rl_repo" not in sys.path:
    sys.path.insert(0, "/opt/trn_rl_repo")

import numpy as np

N_CORES = 8
PAIRS = 4          # (n, h) pairs per core
L = 2048           # query length
S = 2048           # key length
D = 64             # head dim
T = S // 128       # 128-row tiles per pair
LC = 2             # L chunks
LCHUNK = L // LC   # 1024

_PROGRAM_CACHE = {}


def _build_program():
    import concourse.tile as tile
    import concourse.bass as bass
    from concourse import bacc, mybir
    from concourse.bass import ds
    from concourse.masks import make_identity

    f32 = mybir.dt.float32
    f32r = mybir.dt.float32r
    bf16 = mybir.dt.bfloat16
    AF = mybir.ActivationFunctionType

    nc = bacc.Bacc("TRN2", target_bir_lowering=False, debug=False,
                   num_devices=N_CORES)
    q_hbm = nc.dram_tensor("q", [PAIRS, L, D], f32, kind="ExternalInput")
    k_hbm = nc.dram_tensor("k", [PAIRS, S, D], f32, kind="ExternalInput")
    v_hbm = nc.dram_tensor("v", [PAIRS, S, D], f32, kind="ExternalInput")
    t_hbm = nc.dram_tensor("temp", [1, 1], f32, kind="ExternalInput")
    o_hbm = nc.dram_tensor("o", [PAIRS, L, D], f32, kind="ExternalOutput")

    # Per-pair mm1 operand dtype: f32r for pair 0 (PE duty ~100% keeps the
    # HAM un-throttled through the cold-start zone), bf16 after.
    MM1_DT = {0: bf16, 1: bf16, 2: bf16, 3: bf16}

    with tile.TileContext(nc) as tc:
        with (
            tc.tile_pool(name="const", bufs=1) as cpool,
            tc.tile_pool(name="raw", bufs=1) as raw_pool,
            tc.tile_pool(name="io", bufs=2) as io_pool,
            tc.tile_pool(name="work", bufs=2) as work_pool,
            tc.tile_pool(name="small", bufs=4) as small_pool,
            tc.tile_pool(name="pexp", bufs=6) as pexp_pool,
            tc.tile_pool(name="psum1", bufs=2, space="PSUM") as psum1_pool,
            tc.tile_pool(name="psum2", bufs=2, space="PSUM") as psum2_pool,
            tc.tile_pool(name="psmall", bufs=2, space="PSUM") as psmall_pool,
            tc.tile_pool(name="dram", bufs=1, space="DRAM") as dram_pool,
        ):
            identity = cpool.tile([128, 128], f32)
            make_identity(nc, identity[:])
            identity_bf = cpool.tile([128, 128], bf16)
            nc.vector.tensor_copy(identity_bf[:], identity[:])
            identity_r = cpool.tile([128, 128], f32r)
            nc.vector.tensor_copy(identity_r[:], identity[:])

            # Preload the Ln/Exp activation table set off the critical path:
            # no table load between the first norm chain and the softmax Exp.
            tbl = cpool.tile([128, 1], f32)
            nc.vector.memset(tbl[:], 1.0)
            nc.scalar.activation(tbl[:], tbl[:], AF.Ln)

            # Warm-keeper ingredients: regular matmuls count as HAM
            # activity (transpose-mode does not), keeping the PE p-state up.
            scratch_f = cpool.tile([128, 512], f32)
            nc.vector.memset(scratch_f[:], 0.0)
            scratch_b = cpool.tile([128, 512], bf16)
            nc.vector.tensor_copy(scratch_b[:], scratch_f[:])
            scratch_w = cpool.tile([128, 128], bf16)
            nc.vector.memset(scratch_w[:], 0.0)

            def warm(n):
                # fresh pool tiles each time: never pins a psmall slot
                for i in range(n):
                    wk = psmall_pool.tile([128, 512], f32, tag="tp", name="wk")
                    nc.tensor.matmul(wk[:], scratch_w[:], scratch_b[:])

            # HAM warmup while input DMAs stream in.
            warm(12)

            # 1/temp broadcast to [128, 1] (bounce through DRAM for the
            # partition-broadcast DMA).
            t_sb = cpool.tile([1, 1], f32)
            nc.sync.dma_start(t_sb[:], t_hbm.ap())
            rt_sb = cpool.tile([1, 1], f32)
            nc.vector.reciprocal(rt_sb[:], t_sb[:])
            rt_dram = dram_pool.tile([1, 1], f32)
            nc.sync.dma_start(rt_dram[:], rt_sb[:])
            rt_b = cpool.tile([128, 1], f32)
            nc.sync.dma_start(rt_b[:], rt_dram[:].to_broadcast([128, 1]))

            q_raw, k_raw = {}, {}
            v_stage, v_aug = {}, {}

            def dma_qk(p):
                q_raw[p] = raw_pool.tile([128, T, D], f32,
                                         tag=f"qraw{p}", name=f"qraw{p}")
                nc.sync.dma_start(
                    q_raw[p][:],
                    q_hbm.ap()[p].rearrange("(t pp) d -> pp t d", pp=128))
                k_raw[p] = raw_pool.tile([128, T, D], f32,
                                         tag=f"kraw{p}", name=f"kraw{p}")
                nc.sync.dma_start(
                    k_raw[p][:],
                    k_hbm.ap()[p].rearrange("(t pp) d -> pp t d", pp=128))

            def prep_v(p):
                # V with ones column appended; bf16 for mm2.
                v_stage[p] = io_pool.tile([128, T, D + 1], f32, tag="vstage",
                                          name=f"vstage{p}")
                nc.vector.memset(v_stage[p][:, :, D:D + 1], 1.0)
                nc.sync.dma_start(
                    v_stage[p][:, :, 0:D],
                    v_hbm.ap()[p].rearrange("(t pp) d -> pp t d", pp=128))
                v_aug[p] = io_pool.tile([128, T, D + 1], bf16, tag="vaug",
                                        name=f"vaug{p}")
                nc.vector.tensor_copy(v_aug[p][:], v_stage[p][:])

            dma_qk(0)
            prep_v(0)
            for p in range(1, PAIRS):
                dma_qk(p)

            ssq, r_n, rq, rk = {}, {}, {}, {}

            def prep_norm(p):
                # Row sum-of-squares on DVE; rsqrt as exp(-0.5*ln(ssq)).
                ssq[p] = cpool.tile([128, 2, T], f32, name=f"ssq{p}",
                                    tag=f"ssq{p}")
                for i, srct in ((0, q_raw[p]), (1, k_raw[p])):
                    sq = work_pool.tile([128, T, D], f32, tag="sq")
                    nc.vector.tensor_mul(sq[:], srct[:], srct[:])
                    nc.vector.tensor_reduce(
                        ssq[p][:, i, :], sq[:],
                        axis=mybir.AxisListType.X, op=mybir.AluOpType.add)
                r_n[p] = cpool.tile([128, 2, T], f32, name=f"r_n{p}",
                                    tag=f"r_n{p}")
                nc.scalar.activation(ssq[p][:], ssq[p][:], AF.Ln)
                nc.scalar.activation(r_n[p][:], ssq[p][:], AF.Exp, scale=-0.5)
                rq[p] = r_n[p][:, 0, :]
                rk[p] = r_n[p][:, 1, :]
                nc.vector.tensor_scalar_mul(rk[p], rk[p], rt_b[:])

            qnT, knT = {}, {}

            def prep_tp(p):
                # Normalize + PE-transpose to [64, 2048] in the pair's mm1
                # dtype (f32r for pair 0, bf16 after).
                dt = MM1_DT[p]
                ident = identity_r if dt == f32r else identity_bf
                qn = work_pool.tile([128, T, D], dt, tag="qn")
                kn = work_pool.tile([128, T, D], dt, tag="kn")
                for rr, srct, dstt in ((rq[p], q_raw[p], qn),
                                       (rk[p], k_raw[p], kn)):
                    r_b = bass.AP(tensor=rr.tensor, offset=rr.offset,
                                  ap=[rr.ap[0], rr.ap[1], [0, D]])
                    nc.vector.tensor_mul(dstt[:], srct[:], r_b)
                qnT[p] = raw_pool.tile([64, L], dt, tag=f"qnT{p}",
                                       name=f"qnT{p}")
                knT[p] = raw_pool.tile([64, S], dt, tag=f"knT{p}",
                                       name=f"knT{p}")
                for srct, dstt in ((qn, qnT[p]), (kn, knT[p])):
                    for g in range(T // 4):
                        tp = psmall_pool.tile([64, 4, 128], dt, tag="tp")
                        for j in range(4):
                            nc.tensor.transpose(
                                tp[:, j, :], srct[:, 4 * g + j, :], ident)
                        nc.vector.tensor_copy(dstt[:, ds(512 * g, 512)], tp[:])
                    warm(1)  # keep the HAM busy-window alive

            def main_chunk(p, lc):
                ps2 = {}
                for h in range(LCHUNK // 512):
                    ps2[h] = psum2_pool.tile([D + 1, 512], f32, tag="ps2",
                                             name=f"ps2_{h}")
                for st in range(T):
                    ps1 = psum1_pool.tile([128, LCHUNK], f32, tag="ps1")
                    lhs1 = knT[p][:, ds(st * 128, 128)]
                    for h in range(LCHUNK // 512):
                        nc.tensor.matmul(
                            ps1[:, ds(h * 512, 512)], lhs1,
                            qnT[p][:, ds(lc * LCHUNK + h * 512, 512)])
                    pexp = pexp_pool.tile([128, LCHUNK], bf16, tag="pexp")
                    nc.scalar.activation(pexp[:], ps1[:], AF.Exp)
                    lhs2 = v_aug[p][:, st, :]
                    for h in range(LCHUNK // 512):
                        nc.tensor.matmul(
                            ps2[h][:], lhs2,
                            pexp[:, ds(h * 512, 512)],
                            start=(st == 0), stop=(st == T - 1))

                # Epilogue per 512-half; one chunked DMA per half.
                for h in range(LCHUNK // 512):
                    o_sb = work_pool.tile([D + 1, 512], f32, tag="osb")
                    nc.vector.tensor_copy(o_sb[:], ps2[h][:])
                    o_fin = small_pool.tile([128, 4, D], f32, tag="ofin")
                    for j in range(512 // 128):
                        tp = psmall_pool.tile([128, D + 1], f32, tag="tp")
                        nc.tensor.transpose(
                            tp[:], o_sb[:, ds(j * 128, 128)],
                            identity[0:D + 1, 0:D + 1])
                        rcp = small_pool.tile([128, 1], f32, tag="rcp")
                        nc.vector.reciprocal(rcp[:], tp[:, D:D + 1])
                        nc.vector.tensor_scalar_mul(
                            o_fin[:, j, :], tp[:, 0:D], rcp[:])
                    nc.sync.dma_start(
                        o_hbm.ap()[p, ds(lc * LCHUNK + h * 512, 512), :]
                            .rearrange("(j pp) d -> pp j d", pp=128),
                        o_fin[:])

            # ---- Pipelined schedule: pair p+1's prep runs under pair p's
            # main loop; only pair 0's prep is exposed.
            prep_norm(0)
            prep_tp(0)
            prep_norm(1)
            main_chunk(0, 0)
            prep_tp(1)
            prep_v(1)
            main_chunk(0, 1)
            prep_norm(2)
            main_chunk(1, 0)
            prep_tp(2)
            prep_v(2)
            main_chunk(1, 1)
            prep_norm(3)
            main_chunk(2, 0)
            prep_tp(3)
            prep_v(3)
            main_chunk(2, 1)
            main_chunk(3, 0)
            main_chunk(3, 1)

    nc.compile()
    return nc


def _get_program():
    if "nc" not in _PROGRAM_CACHE:
        _PROGRAM_CACHE["nc"] = _build_program()
    return _PROGRAM_CACHE["nc"]


def kernel(queries, keys, values, temp_scale):
    from concourse.bass_utils import run_bass_kernel_spmd

    N, Lq, H, Dh = queries.shape
    assert (N, Lq, H, Dh) == (4, L, 8, D), (N, Lq, H, Dh)

    # [N, L, H, D] -> [N*H, L, D]; core c owns pairs 4c..4c+4.
    def shard(x):
        x = np.ascontiguousarray(
            np.asarray(x, dtype=np.float32).transpose(0, 2, 1, 3)
        ).reshape(N * H, Lq, Dh)
        return [np.ascontiguousarray(x[PAIRS * c:PAIRS * (c + 1)])
                for c in range(N_CORES)]

    qs, ks, vs = shard(queries), shard(keys), shard(values)
    t11 = np.asarray(temp_scale, dtype=np.float32).reshape(1, 1)
    in_maps = [
        {"q": qs[c], "k": ks[c], "v": vs[c], "temp": t11}
        for c in range(N_CORES)
    ]

    nc = _get_program()
    res = run_bass_kernel_spmd(nc, in_maps, core_ids=list(range(N_CORES)))
    if getattr(res, "exec_time_ns", None):
        print(f"HW exec time: {res.exec_time_ns} ns")

    out = np.stack([res.results[c]["o"] for c in range(N_CORES)])  # [8,4,L,D]
    out = out.reshape(N, H, Lq, Dh).transpose(0, 2, 1, 3)          # [N,L,H,D]
    return np.ascontiguousarray(out)


# revision 18
# speedup vs baseline: 1.0565x; 1.0016x over previous
"""Cosine attention kernel for Trainium2, sharded over 8 NeuronCores.

Problem: N=4, L=S=2048, H=8, D=64 fp32.
  q = queries / ||queries||_D ; k = keys / ||keys||_D
  qk = einsum('nlhd,nshd->nlsh', q, k); A = softmax(qk / temp, axis=2)
  out = einsum('nlsh,nshd->nlhd', A, values)

Sharding: the 32 (n, h) pairs are split 4-per-core (data + head parallel).
Each core computes 4 independent 2048x2048 attention problems; no
cross-core communication.

Per-core design: ACT's softmax Exp over the full 2048x2048 score matrix
(128 x [128,1024] PSUM->SBUF activations, ~1.11us each) is the critical
engine; the PE runs just underneath it on all-bf16 matmuls (bf16 streams
at ~226ns/512col vs f32r's ~330 -- the f32 path caps well below the
2.4GHz PE clock).  Normalized Q/K are cast to bf16 (output L2 rel err
~2.5e-3, far inside the 2e-2 gate).

Per-pair prep is software-pipelined under the previous pair's main loop
so only pair 0's prep is exposed at the head:
  - row sum-of-squares + reduce on DVE; rsqrt as exp(-0.5*ln(x)) on ACT
    (Ln and Exp live in the same activation-table set as the softmax Exp
    -- preloaded at start, so no table reloads ever);
  - 1/temp folded into K's row scale;
  - normalize to bf16 (DVE per-partition scalar), PE-transpose 128x64
    tiles (bf16, 1 cyc/col) into QnT/KnT [64, 2048];
  - V loaded with a ones column appended and cast to bf16 (the ones
    column makes mm2's row 64 accumulate the softmax denominator).

Main loop per (pair, 1024-col L-chunk), double-buffered PSUM:
  - P^T[s_tile, l] = KnT_tile^T @ QnT into PSUM [128,1024] (2 bf16 mms)
  - pexp = Exp(P^T) on ACT -> SBUF bf16
  - psum2[h][65, 512] += V_aug[s_tile]^T @ pexp_half (16-step accum)
  - epilogue per 512-half: DVE copy -> PE transpose [65,128] -> [128,65]
    -> DVE reciprocal of the denominator column + per-partition scale ->
    one chunked DMA [128,4,64] per half.

Warm-keeper matmuls (regular bf16 matmuls; transpose-mode does not count
as HAM activity) run while input DMAs stream and between transpose
groups to keep the PE's HAM clock gate from throttling.  Note the HAM on
this silicon sticks at K=4/8 (1.2GHz) for the first ~60-80us regardless
of schedule (known-flaky un-throttle); the schedule tolerates it by
keeping ACT fed continuously.
"""

import sys

if "/opt/trn_rl_repo" not in sys.path:
    sys.path.insert(0, "/opt/trn_rl_repo")

import numpy as np

N_CORES = 8
PAIRS = 4          # (n, h) pairs per core
L = 2048           # query length
S = 2048           # key length
D = 64             # head dim
T = S // 128       # 128-row tiles per pair
LC = 2             # L chunks
LCHUNK = L // LC   # 1024

_PROGRAM_CACHE = {}


def _build_program():
    import concourse.tile as tile
    import concourse.bass as bass
    from concourse import bacc, mybir
    from concourse.bass import ds
    from concourse.masks import make_identity

    f32 = mybir.dt.float32
    f32r = mybir.dt.float32r
    bf16 = mybir.dt.bfloat16
    AF = mybir.ActivationFunctionType

    nc = bacc.Bacc("TRN2", target_bir_lowering=False, debug=False,
                   num_devices=N_CORES)
    q_hbm = nc.dram_tensor("q", [PAIRS, L, D], f32, kind="ExternalInput")
    k_hbm = nc.dram_tensor("k", [PAIRS, S, D], f32, kind="ExternalInput")
    v_hbm = nc.dram_tensor("v", [PAIRS, S, D], f32, kind="ExternalInput")
    t_hbm = nc.dram_tensor("temp", [1, 1], f32, kind="ExternalInput")
    o_hbm = nc.dram_tensor("o", [PAIRS, L, D], f32, kind="ExternalOutput")

    # Per-pair mm1 operand dtype: f32r for pair 0 (PE duty ~100% keeps the
    # HAM un-throttled through the cold-start zone), bf16 after.
    MM1_DT = {0: bf16, 1: bf16, 2: bf16, 3: bf16}

    with tile.TileContext(nc) as tc:
        with (
            tc.tile_pool(name="const", bufs=1) as cpool,
            tc.tile_pool(name="raw", bufs=1) as raw_pool,
            tc.tile_pool(name="io", bufs=2) as io_pool,
            tc.tile_pool(name="work", bufs=2) as work_pool,
            tc.tile_pool(name="small", bufs=4) as small_pool,
            tc.tile_pool(name="pexp", bufs=6) as pexp_pool,
            tc.tile_pool(name="psum1", bufs=2, space="PSUM") as psum1_pool,
            tc.tile_pool(name="psum2", bufs=2, space="PSUM") as psum2_pool,
            tc.tile_pool(name="psmall", bufs=2, space="PSUM") as psmall_pool,
            tc.tile_pool(name="dram", bufs=1, space="DRAM") as dram_pool,
        ):
            identity = cpool.tile([128, 128], f32)
            make_identity(nc, identity[:])
            identity_bf = cpool.tile([128, 128], bf16)
            nc.vector.tensor_copy(identity_bf[:], identity[:])
            identity_r = cpool.tile([128, 128], f32r)
            nc.vector.tensor_copy(identity_r[:], identity[:])

            # Preload the Ln/Exp activation table set off the critical path:
            # no table load between the first norm chain and the softmax Exp.
            tbl = cpool.tile([128, 1], f32)
            nc.vector.memset(tbl[:], 1.0)
            nc.scalar.activation(tbl[:], tbl[:], AF.Ln)

            # Warm-keeper ingredients: regular matmuls count as HAM
            # activity (transpose-mode does not), keeping the PE p-state up.
            scratch_f = cpool.tile([128, 512], f32)
            nc.vector.memset(scratch_f[:], 0.0)
            scratch_b = cpool.tile([128, 512], bf16)
            nc.vector.tensor_copy(scratch_b[:], scratch_f[:])
            scratch_w = cpool.tile([128, 128], bf16)
            nc.vector.memset(scratch_w[:], 0.0)

            def warm(n):
                # fresh pool tiles each time: never pins a psmall slot
                for i in range(n):
                    wk = psmall_pool.tile([128, 512], f32, tag="tp", name="wk")
                    nc.tensor.matmul(wk[:], scratch_w[:], scratch_b[:])

            # HAM warmup while input DMAs stream in.
            warm(12)

            # 1/temp broadcast to [128, 1] (bounce through DRAM for the
            # partition-broadcast DMA).
            t_sb = cpool.tile([1, 1], f32)
            nc.sync.dma_start(t_sb[:], t_hbm.ap())
            rt_sb = cpool.tile([1, 1], f32)
            nc.vector.reciprocal(rt_sb[:], t_sb[:])
            rt_dram = dram_pool.tile([1, 1], f32)
            nc.sync.dma_start(rt_dram[:], rt_sb[:])
            rt_b = cpool.tile([128, 1], f32)
            nc.sync.dma_start(rt_b[:], rt_dram[:].to_broadcast([128, 1]))

            q_raw, k_raw = {}, {}
            v_stage, v_aug = {}, {}

            def dma_qk(p):
                q_raw[p] = raw_pool.tile([128, T, D], f32,
                                         tag=f"qraw{p}", name=f"qraw{p}")
                nc.sync.dma_start(
                    q_raw[p][:],
                    q_hbm.ap()[p].rearrange("(t pp) d -> pp t d", pp=128))
                k_raw[p] = raw_pool.tile([128, T, D], f32,
                                         tag=f"kraw{p}", name=f"kraw{p}")
                nc.sync.dma_start(
                    k_raw[p][:],
                    k_hbm.ap()[p].rearrange("(t pp) d -> pp t d", pp=128))

            def prep_v(p):
                # V with ones column appended; bf16 for mm2.
                v_stage[p] = io_pool.tile([128, T, D + 1], f32, tag="vstage",
                                          name=f"vstage{p}")
                nc.vector.memset(v_stage[p][:, :, D:D + 1], 1.0)
                nc.sync.dma_start(
                    v_stage[p][:, :, 0:D],
                    v_hbm.ap()[p].rearrange("(t pp) d -> pp t d", pp=128))
                v_aug[p] = io_pool.tile([128, T, D + 1], bf16, tag="vaug",
                                        name=f"vaug{p}")
                nc.vector.tensor_copy(v_aug[p][:], v_stage[p][:])

            dma_qk(0)
            prep_v(0)
            for p in range(1, PAIRS):
                dma_qk(p)

            ssq, r_n, rq, rk = {}, {}, {}, {}

            def prep_norm(p):
                # Row sum-of-squares on DVE; rsqrt as exp(-0.5*ln(ssq)).
                ssq[p] = cpool.tile([128, 2, T], f32, name=f"ssq{p}",
                                    tag=f"ssq{p}")
                for i, srct in ((0, q_raw[p]), (1, k_raw[p])):
                    sq = work_pool.tile([128, T, D], f32, tag="sq")
                    nc.vector.tensor_mul(sq[:], srct[:], srct[:])
                    nc.vector.tensor_reduce(
                        ssq[p][:, i, :], sq[:],
                        axis=mybir.AxisListType.X, op=mybir.AluOpType.add)
                r_n[p] = cpool.tile([128, 2, T], f32, name=f"r_n{p}",
                                    tag=f"r_n{p}")
                nc.scalar.activation(ssq[p][:], ssq[p][:], AF.Ln)
                nc.scalar.activation(r_n[p][:], ssq[p][:], AF.Exp, scale=-0.5)
                rq[p] = r_n[p][:, 0, :]
                rk[p] = r_n[p][:, 1, :]
                nc.vector.tensor_scalar_mul(rk[p], rk[p], rt_b[:])

            qnT, knT = {}, {}

            def prep_tp(p):
                # Normalize + PE-transpose to [64, 2048] in the pair's mm1
                # dtype (f32r for pair 0, bf16 after).
                dt = MM1_DT[p]
                ident = identity_r if dt == f32r else identity_bf
                qn = work_pool.tile([128, T, D], dt, tag="qn")
                kn = work_pool.tile([128, T, D], dt, tag="kn")
                for rr, srct, dstt in ((rq[p], q_raw[p], qn),
                                       (rk[p], k_raw[p], kn)):
                    r_b = bass.AP(tensor=rr.tensor, offset=rr.offset,
                                  ap=[rr.ap[0], rr.ap[1], [0, D]])
                    nc.vector.tensor_mul(dstt[:], srct[:], r_b)
                qnT[p] = raw_pool.tile([64, L], dt, tag=f"qnT{p}",
                                       name=f"qnT{p}")
                knT[p] = raw_pool.tile([64, S], dt, tag=f"knT{p}",
                                       name=f"knT{p}")
                for srct, dstt in ((qn, qnT[p]), (kn, knT[p])):
                    for g in range(T // 4):
                        tp = psmall_pool.tile([64, 4, 128], dt, tag="tp")
                        for j in range(4):
                            nc.tensor.transpose(
                                tp[:, j, :], srct[:, 4 * g + j, :], ident)
                        nc.vector.tensor_copy(dstt[:, ds(512 * g, 512)], tp[:])
                    warm(1)  # keep the HAM busy-window alive

            def main_chunk(p, lc):
                ps2 = {}
                for h in range(LCHUNK // 512):
                    ps2[h] = psum2_pool.tile([D + 1, 512], f32, tag="ps2",
                                             name=f"ps2_{h}")
                for st in range(T):
                    ps1 = psum1_pool.tile([128, LCHUNK], f32, tag="ps1")
                    lhs1 = knT[p][:, ds(st * 128, 128)]
                    for h in range(LCHUNK // 512):
                        nc.tensor.matmul(
                            ps1[:, ds(h * 512, 512)], lhs1,
                            qnT[p][:, ds(lc * LCHUNK + h * 512, 512)])
                    pexp = pexp_pool.tile([128, LCHUNK], bf16, tag="pexp")
                    nc.scalar.activation(pexp[:], ps1[:], AF.Exp)
                    lhs2 = v_aug[p][:, st, :]
                    for h in range(LCHUNK // 512):
                        nc.tensor.matmul(
                            ps2[h][:], lhs2,
                            pexp[:, ds(h * 512, 512)],
                            start=(st == 0), stop=(st == T - 1))

                # Epilogue per 512-half; one chunked DMA per half.
                for h in range(LCHUNK // 512):
                    o_sb = work_pool.tile([D + 1, 512], f32, tag="osb")
                    nc.vector.tensor_copy(o_sb[:], ps2[h][:])
                    o_fin = small_pool.tile([128, 4, D], f32, tag="ofin")
                    for j in range(512 // 128):
                        tp = psmall_pool.tile([128, D + 1], f32, tag="tp")
                        nc.tensor.transpose(
                            tp[:], o_sb[:, ds(j * 128, 128)],
                            identity[0:D + 1, 0:D + 1])
                        rcp = small_pool.tile([128, 1], f32, tag="rcp")
                        nc.vector.reciprocal(rcp[:], tp[:, D:D + 1])
                        nc.vector.tensor_scalar_mul(
                            o_fin[:, j, :], tp[:, 0:D], rcp[:])
                    nc.sync.dma_start(
                        o_hbm.ap()[p, ds(lc * LCHUNK + h * 512, 512), :]
                            .rearrange("(j pp) d -> pp j d", pp=128),
                        o_fin[:])

            # ---- Pipelined schedule: pair p+1's prep runs under pair p's
            # main loop; only pair 0's prep is exposed.
            prep_norm(0)
            prep_tp(0)
            prep_norm(1)
            main_chunk(0, 0)
            prep_tp(1)
            prep_v(1)
            main_chunk(0, 1)
            prep_norm(2)
            main_chunk(1, 0)
            prep_tp(2)
            prep_v(2)
            main_chunk(1, 1)
            prep_norm(3)
            main_chunk(2, 0)
            prep_tp(3)
            prep_v(3)
            main_chunk(2, 1)
            main_chunk(3, 0)
            main_chunk(3, 1)

    nc.compile()
    return nc


def _get_program():
    if "nc" not in _PROGRAM_CACHE:
        _PROGRAM_CACHE["nc"] = _build_program()
    return _PROGRAM_CACHE["nc"]


def kernel(queries, keys, values, temp_scale):
    from concourse.bass_utils import run_bass_kernel_spmd

    N, Lq, H, Dh = queries.shape
    assert (N, Lq, H, Dh) == (4, L, 8, D), (N, Lq, H, Dh)

    # [N, L, H, D] -> [N*H, L, D]; core c owns pairs 4c..4c+4.
    def shard(x):
        x = np.ascontiguousarray(
            np.asarray(x, dtype=np.float32).transpose(0, 2, 1, 3)
        ).reshape(N * H, Lq, Dh)
        return [np.ascontiguousarray(x[PAIRS * c:PAIRS * (c + 1)])
                for c in range(N_CORES)]

    qs, ks, vs = shard(queries), shard(keys), shard(values)
    t11 = np.asarray(temp_scale, dtype=np.float32).reshape(1, 1)
    in_maps = [
        {"q": qs[c], "k": ks[c], "v": vs[c], "temp": t11}
        for c in range(N_CORES)
    ]

    nc = _get_program()
    res = run_bass_kernel_spmd(nc, in_maps, core_ids=list(range(N_CORES)))
    if getattr(res, "exec_time_ns", None):
        print(f"HW exec time: {res.exec_time_ns} ns")

    out = np.stack([res.results[c]["o"] for c in range(N_CORES)])  # [8,4,L,D]
    out = out.reshape(N, H, Lq, Dh).transpose(0, 2, 1, 3)          # [N,L,H,D]
    return np.ascontiguousarray(out)
